# revision 4
# baseline (speedup 1.0000x reference)
"""Trainium2 Bass kernel for a single-step attention LSTM decoder.

Computation (matches the reference nn.Decoder):
  x = embedding[word]                          # [H]
  gates = w_ih @ x + b_ih + w_hh @ h0 + b_hh   # [4H]
  i, f, g, o = sigmoid/tanh gate split
  c = f * c0 + i * g ; h = o * tanh(c)
  scores = enc @ h ; attn = softmax(scores)    # [S]
  attn_out = attn @ enc                        # [H]
  logits = W_out @ [h; attn_out] + b_out       # [V]
  logp = log_softmax(logits)

Sharding (8 cores):
  - LSTM output dim: core c computes h[c*128:(c+1)*128] (weight rows sharded)
  - encoder_outputs: sequence-sharded, 1024 timesteps per core
  - W_out / b_out / logits: vocab-sharded (padded to 51200 = 8*6400)
  Cross-core syncs: AllGather(h), AllGather(softmax stats), AllReduce(attn_out),
  AllGather(logsoftmax stats).
"""

import numpy as np

import concourse.bass as bass
import concourse.mybir as mybir
from concourse.bass_utils import run_bass_kernel_spmd
from concourse.tile import TileContext

F32 = mybir.dt.float32
AF = mybir.ActivationFunctionType

H = 1024
S = 8192
V = 50257
NCORES = 8
SC = S // NCORES          # 1024 timesteps per core
NSC = SC // 128           # 8 s-chunks per core
HC = H // 128             # 8 h-chunks
VP = 51200                # padded vocab (8 * 6400)
VC = VP // NCORES         # 6400 vocab rows per core
NVC = VC // 128           # 50 v-chunks per core
K2 = 2 * H                # 2048 contraction for W_out
NK2 = K2 // 128           # 16 k-chunks
NEG_BIG = -1.0e30

_nop_ctr = [0]


def _split_multi_waits(nc):
    """This toolchain's walrus accepts at most one semaphore wait per
    instruction; hoist extra waits onto preceding same-engine no-ops."""
    for _name, bbw in nc.bb_map.items():
        bb = bbw.bb
        new_list = []
        changed = False
        for inst in bb.instructions:
            si = inst.sync_info
            if si is not None and si.on_wait is not None and len(si.on_wait) > 1:
                waits = list(si.on_wait)
                for w in waits[:-1]:
                    _nop_ctr[0] += 1
                    nop = mybir.InstNoOp(
                        name=f"waitnop-{_nop_ctr[0]}", ins=[], outs=[]
                    )
                    nop.engine = inst.engine
                    nop.sync_info = mybir.SyncInfo(on_update=[], on_wait=[w])
                    new_list.append(nop)
                si.on_wait = waits[-1:]
                changed = True
            new_list.append(inst)
        if changed:
            bb.instructions = new_list


def _build_nc(w_bufs=3, w_dma_split=4, apply_split=True):
    nc = bass.Bass()

    # inputs (per core)
    wselT = nc.dram_tensor("wselT", [K2, 512], F32, kind="ExternalInput")
    bias4 = nc.dram_tensor("bias4", [128, 4], F32, kind="ExternalInput")
    cprev = nc.dram_tensor("cprev", [128, 1], F32, kind="ExternalInput")
    xcat = nc.dram_tensor("xcat", [128, 16], F32, kind="ExternalInput")
    encT = nc.dram_tensor("encT", [H, SC], F32, kind="ExternalInput")
    encN = nc.dram_tensor("encN", [SC, H], F32, kind="ExternalInput")
    woutT = nc.dram_tensor("woutT", [K2, VC], F32, kind="ExternalInput")
    boutP = nc.dram_tensor("boutP", [128, NVC], F32, kind="ExternalInput")

    # outputs (per core)
    h_out = nc.dram_tensor("h_out", [128, 1], F32, kind="ExternalOutput")
    c_out = nc.dram_tensor("c_out", [128, 1], F32, kind="ExternalOutput")
    attn_o = nc.dram_tensor("attn_o", [128, NSC], F32, kind="ExternalOutput")
    logp_o = nc.dram_tensor("logp_o", [128, NVC], F32, kind="ExternalOutput")

    # collective bounce buffers (internal DRAM; outputs must be Shared)
    h_ag_in = nc.dram_tensor("h_ag_in", [128], F32)
    h_ag_out = nc.dram_tensor("h_ag_out", [H], F32, addr_space="Shared")
    st1_in = nc.dram_tensor("st1_in", [2], F32)
    st1_out = nc.dram_tensor("st1_out", [2 * NCORES], F32, addr_space="Shared")
    ao_in = nc.dram_tensor("ao_in", [H], F32)
    ao_out = nc.dram_tensor("ao_out", [H], F32, addr_space="Shared")
    st2_in = nc.dram_tensor("st2_in", [2], F32)
    st2_out = nc.dram_tensor("st2_out", [2 * NCORES], F32, addr_space="Shared")

    groups = [list(range(NCORES))]

    with TileContext(nc) as tc:
        with (
            tc.tile_pool(name="consts", bufs=1) as consts,
            tc.tile_pool(name="wsel", bufs=1) as wsel_pool,
            tc.tile_pool(name="enc", bufs=1) as enc_pool,
            tc.tile_pool(name="wstrip", bufs=w_bufs) as wstrip_pool,
            tc.tile_pool(name="small", bufs=1) as small,
            tc.tile_pool(name="psA", bufs=2, space="PSUM") as psA,
            tc.tile_pool(name="psS", bufs=2, space="PSUM") as psS,
            tc.tile_pool(name="psL", bufs=1, space="PSUM") as psL,
        ):
            # ---- constants ----
            ones_row = consts.tile([1, 128], F32)    # for scalar->partition bcast
            nc.vector.memset(ones_row[:, :], 1.0)
            ones_col = consts.tile([128, 1], F32)    # for cross-partition sums
            nc.vector.memset(ones_col[:, :], 1.0)
            identity = consts.tile([128, 128], F32)
            from concourse.masks import make_identity
            make_identity(nc, identity)

            def bcast(scalar_ap, out_sb_tile):
                """broadcast [1,1] scalar across 128 partitions -> [128,1] sbuf"""
                ps = psS.tile([128, 1], F32, tag="psS")
                nc.tensor.matmul(ps[:, :], ones_row[:, :], scalar_ap, start=True, stop=True)
                nc.vector.tensor_copy(out_sb_tile[:, :], ps[:, :])

            def cross_part_sum(in_col_ap, out_sb_tile):
                """[128,1] per-partition values -> [1,1] total (via ones matmul)"""
                ps = psS.tile([1, 1], F32, tag="psS")
                nc.tensor.matmul(ps[:, :], in_col_ap, ones_col[:, :], start=True, stop=True)
                nc.vector.tensor_copy(out_sb_tile[:, :], ps[:, :])

            def cross_part_max(in_col_ap, out_sb_tile, tmp_tag):
                """[128,1] per-partition values -> [1,1] max (transpose+reduce)"""
                ps = psS.tile([1, 128], F32, tag="psS")
                nc.tensor.transpose(ps[:, :], in_col_ap, identity[:, :])
                row = small.tile([1, 128], F32, tag=f"cpm_{tmp_tag}")
                nc.vector.tensor_copy(row[:, :], ps[:, :])
                nc.vector.tensor_reduce(
                    out_sb_tile[:, :], row[:, :], axis=mybir.AxisListType.X,
                    op=mybir.AluOpType.max,
                )

            # ---- phase A: LSTM gate GEMV ----
            xcat_sb = consts.tile([128, 16], F32)
            nc.sync.dma_start(out=xcat_sb[:, :], in_=xcat[:, :])
            bias_sb = consts.tile([128, 4], F32)
            nc.sync.dma_start(out=bias_sb[:, :], in_=bias4[:, :])
            cprev_sb = consts.tile([128, 1], F32)
            nc.sync.dma_start(out=cprev_sb[:, :], in_=cprev[:, :])

            wsel_sb = wsel_pool.tile([128, NK2 * 512], F32)
            for k in range(NK2):
                nc.sync.dma_start(
                    out=wsel_sb[:, k * 512:(k + 1) * 512],
                    in_=wselT[k * 128:(k + 1) * 128, :],
                )

            psum_g = psA.tile([128, 4], F32, tag="psA")
            for m in range(4):
                for k in range(NK2):
                    nc.tensor.matmul(
                        psum_g[:, m:m + 1],
                        wsel_sb[:, k * 512 + m * 128: k * 512 + (m + 1) * 128],
                        xcat_sb[:, k:k + 1],
                        start=(k == 0),
                        stop=(k == NK2 - 1),
                    )

            gates = small.tile([128, 4], F32)
            nc.vector.tensor_add(gates[:, :], psum_g[:, :], bias_sb[:, :])
            acts = small.tile([128, 4], F32)
            nc.scalar.activation(acts[:, 0:2], gates[:, 0:2], AF.Sigmoid)
            nc.scalar.activation(acts[:, 3:4], gates[:, 3:4], AF.Sigmoid)
            nc.scalar.activation(acts[:, 2:3], gates[:, 2:3], AF.Tanh)

            c_new = small.tile([128, 1], F32)
            nc.vector.tensor_mul(c_new[:, :], acts[:, 1:2], cprev_sb[:, :])   # f*c0
            ig = small.tile([128, 1], F32)
            nc.vector.tensor_mul(ig[:, :], acts[:, 0:1], acts[:, 2:3])        # i*g
            nc.vector.tensor_add(c_new[:, :], c_new[:, :], ig[:, :])
            nc.sync.dma_start(out=c_out[:, :], in_=c_new[:, :])

            tanh_c = small.tile([128, 1], F32)
            nc.scalar.activation(tanh_c[:, :], c_new[:, :], AF.Tanh)
            h_slice = small.tile([128, 1], F32)
            nc.vector.tensor_mul(h_slice[:, :], acts[:, 3:4], tanh_c[:, :])
            nc.sync.dma_start(out=h_out[:, :], in_=h_slice[:, :])

            # ---- phase B: AllGather h ----
            nc.sync.dma_start(out=h_ag_in.ap().rearrange("(p o) -> p o", o=1), in_=h_slice[:, :])
            nc.gpsimd.collective_compute(
                "AllGather", mybir.AluOpType.bypass,
                ins=[h_ag_in[:]], outs=[h_ag_out[:]], replica_groups=groups,
            )
            h_sb = consts.tile([128, HC], F32)
            nc.sync.dma_start(
                out=h_sb[:, :], in_=h_ag_out.ap().rearrange("(c p) -> p c", p=128)
            )

            # ---- phase C: attention scores (encT stationary, h chunks moving) ----
            encT_sb = enc_pool.tile([128, HC * SC], F32, tag="encT")
            for k in range(HC):
                half = SC // 2
                for d in range(2):
                    nc.sync.dma_start(
                        out=encT_sb[:, k * SC + d * half: k * SC + (d + 1) * half],
                        in_=encT[k * 128:(k + 1) * 128, d * half:(d + 1) * half],
                    )
            encN_sb = enc_pool.tile([128, NSC * H], F32, tag="encN")
            for sj in range(NSC):
                half = H // 2
                for d in range(2):
                    nc.sync.dma_start(
                        out=encN_sb[:, sj * H + d * half: sj * H + (d + 1) * half],
                        in_=encN[sj * 128:(sj + 1) * 128, d * half:(d + 1) * half],
                    )

            psum_s = psA.tile([128, NSC], F32, tag="psA")
            for sj in range(NSC):
                for k in range(HC):
                    nc.tensor.matmul(
                        psum_s[:, sj:sj + 1],
                        encT_sb[:, k * SC + sj * 128: k * SC + (sj + 1) * 128],
                        h_sb[:, k:k + 1],
                        start=(k == 0),
                        stop=(k == HC - 1),
                    )
            scores = small.tile([128, NSC], F32)
            nc.vector.tensor_copy(scores[:, :], psum_s[:, :])

            # ---- phase D: local softmax stats ----
            rowmax = small.tile([128, 1], F32)
            nc.vector.tensor_reduce(
                rowmax[:, :], scores[:, :], axis=mybir.AxisListType.X,
                op=mybir.AluOpType.max,
            )
            m_loc = small.tile([1, 1], F32)
            cross_part_max(rowmax[:, :], m_loc, "m1")
            negm = small.tile([1, 1], F32)
            nc.vector.tensor_scalar_mul(negm[:, :], m_loc[:, :], -1.0)
            negm_b = small.tile([128, 1], F32)
            bcast(negm[:, :], negm_b)

            E = small.tile([128, NSC], F32)
            Esum = small.tile([128, 1], F32)
            nc.scalar.activation(
                E[:, :], scores[:, :], AF.Exp, bias=negm_b[:, 0:1],
                accum_out=Esum[:, :],
            )
            S_loc = small.tile([1, 1], F32)
            cross_part_sum(Esum[:, :], S_loc)

            st1_sb = small.tile([1, 2], F32)
            nc.vector.tensor_copy(st1_sb[:, 0:1], m_loc[:, :])
            nc.vector.tensor_copy(st1_sb[:, 1:2], S_loc[:, :])
            nc.sync.dma_start(out=st1_in.ap().rearrange("(o f) -> o f", o=1), in_=st1_sb[:, :])
            nc.gpsimd.collective_compute(
                "AllGather", mybir.AluOpType.bypass,
                ins=[st1_in[:]], outs=[st1_out[:]], replica_groups=groups,
            )

            # ---- phase E: global softmax normalization ----
            st1a = small.tile([1, 2 * NCORES], F32)
            nc.sync.dma_start(
                out=st1a[:, :], in_=st1_out.ap().rearrange("(o f) -> o f", o=1)
            )
            m_all = st1a[:, 0:2 * NCORES:2]
            s_all = st1a[:, 1:2 * NCORES:2]
            M_g = small.tile([1, 1], F32)
            nc.vector.tensor_reduce(
                M_g[:, :], m_all, axis=mybir.AxisListType.X, op=mybir.AluOpType.max
            )
            negM = small.tile([1, 1], F32)
            nc.vector.tensor_scalar_mul(negM[:, :], M_g[:, :], -1.0)
            em = small.tile([1, NCORES], F32)
            nc.scalar.activation(em[:, :], m_all, AF.Exp, bias=negM[:, 0:1])
            wz = small.tile([1, NCORES], F32)
            nc.vector.tensor_mul(wz[:, :], em[:, :], s_all)
            Z_g = small.tile([1, 1], F32)
            nc.vector.tensor_reduce(
                Z_g[:, :], wz[:, :], axis=mybir.AxisListType.X, op=mybir.AluOpType.add
            )
            rZ = small.tile([1, 1], F32)
            nc.vector.reciprocal(rZ[:, :], Z_g[:, :])
            # alpha = exp(m_loc - M) / Z  (uses this core's local max)
            alpha = small.tile([1, 1], F32)
            nc.scalar.activation(alpha[:, :], m_loc[:, :], AF.Exp, bias=negM[:, 0:1])
            nc.vector.tensor_mul(alpha[:, :], alpha[:, :], rZ[:, :])
            alpha_b = small.tile([128, 1], F32)
            bcast(alpha[:, :], alpha_b)

            attnP = small.tile([128, NSC], F32)
            nc.vector.tensor_scalar_mul(attnP[:, :], E[:, :], alpha_b[:, 0:1])
            nc.sync.dma_start(out=attn_o[:, :], in_=attnP[:, :])

            # ---- phase F: attn_out partials (enc natural stationary) ----
            psum_ao = psA.tile([128, HC], F32, tag="psA")
            for hc in range(HC):
                for sj in range(NSC):
                    nc.tensor.matmul(
                        psum_ao[:, hc:hc + 1],
                        encN_sb[:, sj * H + hc * 128: sj * H + (hc + 1) * 128],
                        attnP[:, sj:sj + 1],
                        start=(sj == 0),
                        stop=(sj == NSC - 1),
                    )
            ao_sb = small.tile([128, HC], F32)
            nc.vector.tensor_copy(ao_sb[:, :], psum_ao[:, :])
            nc.sync.dma_start(
                out=ao_in.ap().rearrange("(c p) -> p c", p=128), in_=ao_sb[:, :]
            )
            nc.gpsimd.collective_compute(
                "AllReduce", mybir.AluOpType.add,
                ins=[ao_in[:]], outs=[ao_out[:]], replica_groups=groups,
            )
            aofull = consts.tile([128, HC], F32)
            nc.sync.dma_start(
                out=aofull[:, :], in_=ao_out.ap().rearrange("(c p) -> p c", p=128)
            )

            # ---- phase G: big vocab GEMV (W_outT stationary, comb moving) ----
            psum_L = psL.tile([128, NVC], F32)
            for k in range(NK2):
                wk = wstrip_pool.tile([128, VC], F32, tag="wk")
                step = VC // w_dma_split
                for d in range(w_dma_split):
                    nc.sync.dma_start(
                        out=wk[:, d * step:(d + 1) * step],
                        in_=woutT[k * 128:(k + 1) * 128, d * step:(d + 1) * step],
                    )
                comb_k = h_sb[:, k:k + 1] if k < HC else aofull[:, k - HC:k - HC + 1]
                for j in range(NVC):
                    # NB: start=True clears has_written for the WHOLE bank, so
                    # with k-outer/j-inner interleaved column groups only the
                    # very first matmul of the bank may carry start=True.
                    nc.tensor.matmul(
                        psum_L[:, j:j + 1],
                        wk[:, j * 128:(j + 1) * 128],
                        comb_k,
                        start=(k == 0 and j == 0),
                        stop=(k == NK2 - 1),
                        skip_group_check=True,
                    )

            bout_sb = small.tile([128, NVC], F32)
            nc.sync.dma_start(out=bout_sb[:, :], in_=boutP[:, :])
            logits = small.tile([128, NVC], F32)
            nc.vector.tensor_add(logits[:, :], psum_L[:, :], bout_sb[:, :])

            # ---- phase H: log-softmax over vocab ----
            rmax2 = small.tile([128, 1], F32)
            nc.vector.tensor_reduce(
                rmax2[:, :], logits[:, :], axis=mybir.AxisListType.X,
                op=mybir.AluOpType.max,
            )
            m2_loc = small.tile([1, 1], F32)
            cross_part_max(rmax2[:, :], m2_loc, "m2")
            negm2 = small.tile([1, 1], F32)
            nc.vector.tensor_scalar_mul(negm2[:, :], m2_loc[:, :], -1.0)
            negm2_b = small.tile([128, 1], F32)
            bcast(negm2[:, :], negm2_b)

            E2 = small.tile([128, NVC], F32)
            E2sum = small.tile([128, 1], F32)
            nc.scalar.activation(
                E2[:, :], logits[:, :], AF.Exp, bias=negm2_b[:, 0:1],
                accum_out=E2sum[:, :],
            )
            S2_loc = small.tile([1, 1], F32)
            cross_part_sum(E2sum[:, :], S2_loc)

            st2_sb = small.tile([1, 2], F32)
            nc.vector.tensor_copy(st2_sb[:, 0:1], m2_loc[:, :])
            nc.vector.tensor_copy(st2_sb[:, 1:2], S2_loc[:, :])
            nc.sync.dma_start(out=st2_in.ap().rearrange("(o f) -> o f", o=1), in_=st2_sb[:, :])
            nc.gpsimd.collective_compute(
                "AllGather", mybir.AluOpType.bypass,
                ins=[st2_in[:]], outs=[st2_out[:]], replica_groups=groups,
            )
            st2a = small.tile([1, 2 * NCORES], F32)
            nc.sync.dma_start(
                out=st2a[:, :], in_=st2_out.ap().rearrange("(o f) -> o f", o=1)
            )
            m2_all = st2a[:, 0:2 * NCORES:2]
            s2_all = st2a[:, 1:2 * NCORES:2]
            M2_g = small.tile([1, 1], F32)
            nc.vector.tensor_reduce(
                M2_g[:, :], m2_all, axis=mybir.AxisListType.X, op=mybir.AluOpType.max
            )
            negM2 = small.tile([1, 1], F32)
            nc.vector.tensor_scalar_mul(negM2[:, :], M2_g[:, :], -1.0)
            em2 = small.tile([1, NCORES], F32)
            nc.scalar.activation(em2[:, :], m2_all, AF.Exp, bias=negM2[:, 0:1])
            wz2 = small.tile([1, NCORES], F32)
            nc.vector.tensor_mul(wz2[:, :], em2[:, :], s2_all)
            Z2_g = small.tile([1, 1], F32)
            nc.vector.tensor_reduce(
                Z2_g[:, :], wz2[:, :], axis=mybir.AxisListType.X,
                op=mybir.AluOpType.add,
            )
            logZ2 = small.tile([1, 1], F32)
            nc.scalar.activation(logZ2[:, :], Z2_g[:, :], AF.Ln)
            # offset = M2 + logZ2 ; logp = logits - offset
            negoff = small.tile([1, 1], F32)
            nc.vector.tensor_add(negoff[:, :], M2_g[:, :], logZ2[:, :])
            nc.vector.tensor_scalar_mul(negoff[:, :], negoff[:, :], -1.0)
            negoff_b = small.tile([128, 1], F32)
            bcast(negoff[:, :], negoff_b)

            logp = small.tile([128, NVC], F32)
            nc.vector.tensor_scalar_add(logp[:, :], logits[:, :], negoff_b[:, 0:1])
            nc.sync.dma_start(out=logp_o[:, :], in_=logp[:, :])

    if apply_split:
        _split_multi_waits(nc)
    return nc


_NC_CACHE = {}


def _get_nc():
    if "nc" not in _NC_CACHE:
        _NC_CACHE["nc"] = _build_nc()
    return _NC_CACHE["nc"]


def _prep_in_maps(word_input, h0, c0, encoder_outputs, embedding,
                  w_ih, w_hh, b_ih, b_hh, W_out, b_out):
    idx = int(np.asarray(word_input).reshape(-1)[0])
    x = np.asarray(embedding, np.float32)[idx]             # [H]
    h_prev = np.asarray(h0, np.float32).reshape(H)
    c_prev = np.asarray(c0, np.float32).reshape(H)
    enc = np.ascontiguousarray(
        np.asarray(encoder_outputs, np.float32).reshape(S, H)
    )
    w_ih = np.asarray(w_ih, np.float32)
    w_hh = np.asarray(w_hh, np.float32)
    bias = (np.asarray(b_ih, np.float32) + np.asarray(b_hh, np.float32))  # [4H]
    W_out = np.asarray(W_out, np.float32)
    b_out = np.asarray(b_out, np.float32)

    xcat = np.concatenate([x, h_prev])                     # [2048]
    xcat_pm = np.ascontiguousarray(xcat.reshape(16, 128).T)

    b_pad = np.full(VP, NEG_BIG, np.float32)
    b_pad[:V] = b_out

    in_maps = []
    for c in range(NCORES):
        rows = np.concatenate(
            [np.arange(b * H + c * 128, b * H + (c + 1) * 128) for b in range(4)]
        )
        wsel = np.concatenate([w_ih[rows], w_hh[rows]], axis=1)   # [512, 2048]
        wselT = np.ascontiguousarray(wsel.T)                      # [2048, 512]
        bias4 = np.ascontiguousarray(bias[rows].reshape(4, 128).T)  # [128, 4]
        cprev = np.ascontiguousarray(c_prev[c * 128:(c + 1) * 128][:, None])

        enc_c = enc[c * SC:(c + 1) * SC]                          # [1024, 1024]
        encT = np.ascontiguousarray(enc_c.T)

        lo, hi = c * VC, (c + 1) * VC
        w_shard = np.zeros((VC, K2), np.float32)
        real = min(hi, V) - lo
        if real > 0:
            w_shard[:real] = W_out[lo:lo + real]
        woutT = np.ascontiguousarray(w_shard.T)                   # [2048, 6400]
        boutP = np.ascontiguousarray(b_pad[lo:hi].reshape(NVC, 128).T)

        in_maps.append({
            "wselT": wselT,
            "bias4": bias4,
            "cprev": cprev,
            "xcat": xcat_pm,
            "encT": encT,
            "encN": enc_c,
            "woutT": woutT,
            "boutP": boutP,
        })
    return in_maps


def kernel(word_input, h0, c0, encoder_outputs, embedding,
           w_ih, w_hh, b_ih, b_hh, W_out, b_out):
    nc = _get_nc()
    in_maps = _prep_in_maps(word_input, h0, c0, encoder_outputs, embedding,
                            w_ih, w_hh, b_ih, b_hh, W_out, b_out)
    res = run_bass_kernel_spmd(nc, in_maps, list(range(NCORES)))

    h_parts, c_parts, attn_parts, logp_parts = [], [], [], []
    for c in range(NCORES):
        r = res.results[c]
        h_parts.append(r["h_out"][:, 0])
        c_parts.append(r["c_out"][:, 0])
        attn_parts.append(r["attn_o"].T.reshape(-1))     # [1024]
        logp_parts.append(r["logp_o"].T.reshape(-1))     # [6400]

    h_full = np.concatenate(h_parts).astype(np.float32)
    c_full = np.concatenate(c_parts).astype(np.float32)
    attn_full = np.concatenate(attn_parts).astype(np.float32)[:S]
    logp_full = np.concatenate(logp_parts).astype(np.float32)[:V]

    logp_ret = logp_full[None, :]
    hidden = (h_full[None, None, :], c_full[None, None, :])
    attn_ret = attn_full[:, None]
    return (logp_ret, hidden, attn_ret)
